# revision 17
# baseline (speedup 1.0000x reference)
"""Distributed causal multi-head attention for one TRN2 chip (8 NeuronCores).

Sharding: batch (2) x head-groups (4 heads/core) -> 8 cores.
Core c handles batch c//4, heads [ (c%4)*4 , (c%4)*4+4 ).
Per core: QKV projections for its 4 heads, flash-style causal attention
with scores kept transposed (S^T = K @ Q^T) so the PV product needs no
transposes; V is augmented with a ones column so the softmax denominators
fall out of the same matmul (row 64 of each head's O^T psum).  Then an
AllGather of the attention output (pre-Wo, 4-core group = one batch) and
a column-sliced output projection.  Host assembles the 8 column/batch
shards.  Compute dtype bf16 (PSUM accumulation fp32), softmax in fp32.

Scheduling: the attention loop keeps the in-order PE queue saturated by
interleaving KT/V projections for later chunks and the AllGather-gated
output projections as queued work items (so the HAM clock gate never
re-throttles).  Chunk 0+1 share one AllGather (it completes under chunk
2's attention); the last chunk's AllGather is split per head-pair so the
first half overlaps pass B and the output projection can start on half
the k-tiles while the second half is still in flight.
"""

import sys
from collections import deque

import numpy as np

sys.path.insert(0, "/opt/trn_rl_repo")

import concourse.bass as bass  # noqa: E402
import concourse.bacc as bacc  # noqa: E402
import concourse.tile as tile  # noqa: E402
import concourse.mybir as mybir  # noqa: E402

F32 = mybir.dt.float32
BF16 = mybir.dt.bfloat16
ActFn = mybir.ActivationFunctionType

P = 128          # partition dim
CHUNK = 512      # i-chunk (matmul moving free dim, one psum bank of fp32)
DH = 64          # head dim
HPC = 4          # heads per core
HS = HPC * DH    # 256 per-core inner slice
DHA = DH + 1     # augmented head dim (ones column for softmax sums)
INNER = 1024     # total inner dim (16 heads x 64)
N_CORES = 8
GROUPS = [[0, 1, 2, 3], [4, 5, 6, 7]]


def build_nc(seq=2048, dim=1024, n_cores=N_CORES, groups=GROUPS, compile=True):
    """Build the SPMD Bass graph (identical on all cores)."""
    nch = seq // CHUNK          # i-chunks
    jpc = CHUNK // P            # j-tiles per chunk (4)
    njt = seq // P              # j-tiles
    nk = dim // P               # feature k-tiles
    nko = INNER // P            # inner k-tiles for the output projection
    grp = len(groups[0])        # replica group size (4)

    nc = bacc.Bacc("TRN2", target_bir_lowering=False, debug=False,
                   enable_asserts=False, num_devices=n_cores)

    xT = nc.dram_tensor("xT", [dim, seq], BF16, kind="ExternalInput").ap()
    wq = nc.dram_tensor("wq", [dim, HS], BF16, kind="ExternalInput").ap()
    wk = nc.dram_tensor("wk", [dim, HS], BF16, kind="ExternalInput").ap()
    wv = nc.dram_tensor("wv", [dim, HS], BF16, kind="ExternalInput").ap()
    wo = nc.dram_tensor("wo", [INNER, HS], BF16, kind="ExternalInput").ap()
    mask_c = nc.dram_tensor("mask_c", [P, P], BF16, kind="ExternalInput").ap()
    outT = nc.dram_tensor("outT", [HS, seq], F32, kind="ExternalOutput").ap()

    with tile.TileContext(nc) as tc:
        with tc.tile_pool(name="sb", bufs=1) as sb, \
             tc.tile_pool(name="ps", bufs=1, space="PSUM") as ps, \
             tc.tile_pool(name="dram", bufs=1, space="DRAM") as dram:

            # ---- load inputs ----
            xt = [sb.tile([P, seq], BF16, tag=f"xt{k}", name=f"xt{k}")
                  for k in range(nk)]
            wq_sb = [sb.tile([P, HS], BF16, tag=f"wq{k}", name=f"wq{k}")
                     for k in range(nk)]
            wk_sb = [sb.tile([P, HS], BF16, tag=f"wk{k}", name=f"wk{k}")
                     for k in range(nk)]
            wv_sb = [sb.tile([P, HS], BF16, tag=f"wv{k}", name=f"wv{k}")
                     for k in range(nk)]
            wo_sb = [sb.tile([P, HS], BF16, tag=f"wo{k}", name=f"wo{k}")
                     for k in range(nko)]
            mask_sb = sb.tile([P, P], BF16, tag="mask", name="mask")

            # inputs spread across engine DMA queues so the loads issue
            # in parallel (the sync queue alone serializes ~40 descriptors)
            for k in range(nk):
                nc.sync.dma_start(xt[k][:], xT[k * P:(k + 1) * P, :])
                nc.scalar.dma_start(wq_sb[k][:], wq[k * P:(k + 1) * P, :])
                nc.scalar.dma_start(wk_sb[k][:], wk[k * P:(k + 1) * P, :])
                nc.gpsimd.dma_start(wv_sb[k][:], wv[k * P:(k + 1) * P, :])
            for k in range(nko):
                nc.gpsimd.dma_start(wo_sb[k][:], wo[k * P:(k + 1) * P, :])
            nc.gpsimd.dma_start(mask_sb[:], mask_c[:])

            # warm up the collectives firmware while QKV runs
            warm_in = dram.tile([P, 4], BF16, tag="warm_i", name="warm_i")
            warm_out = dram.tile([grp * P, 4], BF16,
                                 tag="warm_o", name="warm_o")
            nc.sync.dma_start(warm_in[:], mask_c[0:P, 0:4])
            nc.gpsimd.collective_compute(
                "AllGather", mybir.AluOpType.bypass, replica_groups=groups,
                ins=[warm_in.opt()], outs=[warm_out.opt()])

            # persistent QKV results
            qt_sb = [sb.tile([P, seq], BF16, tag=f"qt{p}", name=f"qt{p}")
                     for p in range(2)]
            kt_sb = [sb.tile([P, seq], BF16, tag=f"kt{p}", name=f"kt{p}")
                     for p in range(2)]
            v_sb = [sb.tile([P, HPC * DHA], BF16, tag=f"v{j}", name=f"v{j}")
                    for j in range(njt)]
            ot_sb = [sb.tile([P, seq], BF16, tag=f"ot{p}", name=f"ot{p}")
                     for p in range(2)]

            # ---- interleavable work items (each emits one psum group) ----
            def emit_kt(pair, ch):
                pt = ps.tile([P, CHUNK], F32, tag="misc",
                             name=f"ktps{pair}_{ch}", bufs=2)
                for k in range(nk):
                    nc.tensor.matmul(
                        pt[:], lhsT=wk_sb[k][:, pair * P:(pair + 1) * P],
                        rhs=xt[k][:, ch * CHUNK:(ch + 1) * CHUNK],
                        start=(k == 0), stop=(k == nk - 1))
                nc.scalar.activation(
                    kt_sb[pair][:, ch * CHUNK:(ch + 1) * CHUNK], pt[:],
                    ActFn.Copy)

            def emit_v(jt):
                pt = ps.tile([P, HS], F32, tag="misc",
                             name=f"vps{jt}", bufs=2)
                for k in range(nk):
                    nc.tensor.matmul(
                        pt[:], lhsT=xt[k][:, jt * P:(jt + 1) * P],
                        rhs=wv_sb[k][:],
                        start=(k == 0), stop=(k == nk - 1))
                nc.scalar.activation(
                    v_sb[jt].rearrange("p (h d) -> p h d", h=HPC)[:, :, 0:DH],
                    pt.rearrange("p (h d) -> p h d", h=HPC), ActFn.Copy)
                nc.vector.memset(
                    v_sb[jt].rearrange("p (h d) -> p h d", h=HPC)[:, :, DH:DHA],
                    1.0)

            def emit_proj(ci, m, slices, korder, op_ps=None, evac=True):
                # transposed output block: outT[m*128:(m+1)*128, chunk ci]
                # = Wo[:, m-slice].T @ attT[:, chunk] over the k-tiles in
                # `korder` (a partial pass keeps op_ps alive).  The moving
                # operand is the gathered attT tile (N=512, half the
                # matmuls of the untransposed form); the host untransposes.
                c0 = ci * CHUNK
                first = op_ps is None
                if first:
                    op_ps = ps.tile([P, CHUNK], F32, tag="misc",
                                    name=f"op{ci}_{m}", bufs=2)
                for n, k in enumerate(korder):
                    ag_t, coff = slices[k]
                    nc.tensor.matmul(
                        op_ps[:],
                        lhsT=wo_sb[k][:, m * P:(m + 1) * P],
                        rhs=ag_t[:, coff:coff + CHUNK],
                        start=(first and n == 0),
                        stop=(evac and n == len(korder) - 1))
                if not evac:
                    return op_ps
                o_sb = sb.tile([P, CHUNK], F32, tag="osb",
                               name=f"o{ci}_{m}", bufs=2)
                nc.vector.tensor_copy(o_sb[:], op_ps[:])
                nc.sync.dma_start(
                    outT[m * P:(m + 1) * P, c0:c0 + CHUNK], o_sb[:])
                return None

            work_early = deque()   # KT/V for future chunks (not gated)
            work_late = deque()    # output projections (gated on AllGather)

            def pop_work(late_ok):
                if work_early:
                    work_early.popleft()()
                elif late_ok and work_late:
                    work_late.popleft()()

            def emit_ag_pair(ci, pair):
                # half AllGather (one head pair) of the chunk `ci` — fired
                # right after that pair's normalize, so pair A overlaps the
                # second attention pass and both stay small (cheap on CC).
                # The gathered k-tiles (2r+pair) load right behind it.
                c0 = ci * CHUNK
                bounce_in = dram.tile([P, CHUNK], BF16, tag=f"binh{pair}",
                                      name=f"binh{ci}_{pair}", bufs=2)
                bounce_out = dram.tile([grp * P, CHUNK], BF16,
                                       tag=f"bouth{pair}",
                                       name=f"bouth{ci}_{pair}", bufs=2)
                nc.sync.dma_start(bounce_in[:], ot_sb[pair][:, c0:c0 + CHUNK])
                nc.gpsimd.collective_compute(
                    "AllGather", mybir.AluOpType.bypass,
                    replica_groups=groups,
                    ins=[bounce_in.opt()], outs=[bounce_out.opt()])
                tiles = {}
                for r in range(grp):
                    k = 2 * r + pair
                    t = sb.tile([P, CHUNK], BF16, tag=f"ag{k}",
                                name=f"ag{ci}_{k}", bufs=2)
                    nc.sync.dma_start(t[:], bounce_out[r * P:(r + 1) * P, :])
                    tiles[k] = t
                return tiles

            # ---- upfront projections: all of Q, chunk-0 K, chunk-0 V ----
            # Q is k-outer / weight-stationary so it pipelines with the xT
            # DMAs; two chunks share one 2-bank psum tile.
            for pair in range(2):
                for chh in range(0, nch, 2):
                    wch = min(2, nch - chh)
                    pt = ps.tile([P, wch * CHUNK], F32, tag="s2",
                                 name=f"qps{pair}_{chh}", bufs=2)
                    for k in range(nk):
                        for c in range(wch):
                            nc.tensor.matmul(
                                pt[:, c * CHUNK:(c + 1) * CHUNK],
                                lhsT=wq_sb[k][:, pair * P:(pair + 1) * P],
                                rhs=xt[k][:, (chh + c) * CHUNK:
                                           (chh + c + 1) * CHUNK],
                                start=(k == 0), stop=(k == nk - 1))
                    nc.scalar.activation(
                        qt_sb[pair][:, chh * CHUNK:(chh + wch) * CHUNK],
                        pt[:], ActFn.Copy)
            for pair in range(2):
                emit_kt(pair, 0)
            for jt in range(jpc):
                emit_v(jt)

            # ---- attention chunks ----
            last_parts = {}
            for ci in range(nch):
                jt_end = jpc * (ci + 1)
                c0 = ci * CHUNK
                last = ci == nch - 1

                if ci + 1 < nch:
                    for pair in range(2):
                        work_early.append(
                            lambda pair=pair, ch=ci + 1: emit_kt(pair, ch))
                    for jt in range(jpc * (ci + 1), jpc * (ci + 2)):
                        work_early.append(lambda jt=jt: emit_v(jt))

                for hpass in range(2):
                    # heads 2*hpass, 2*hpass+1  (== head pair `hpass`)
                    ot_ps = [ps.tile([DHA, CHUNK], F32, tag=f"ot{h2}",
                                     name=f"ot{ci}_{hpass}_{h2}", bufs=1)
                             for h2 in range(2)]
                    for jt in range(jt_end):
                        rel = max(0, (jt - jpc * ci)) * P
                        diag = jt >= jpc * ci

                        s2 = ps.tile([P, 2 * CHUNK], F32, tag="s2",
                                     name=f"s{ci}_{hpass}_{jt}", bufs=2)
                        es = sb.tile([P, 2 * CHUNK], BF16, tag="es",
                                     name=f"es{ci}_{hpass}_{jt}", bufs=3)

                        for h2 in range(2):
                            # S^T tile = K_h @ Q_h^T (row-tiled, K=64)
                            nc.tensor.matmul(
                                s2[:, h2 * CHUNK + rel:(h2 + 1) * CHUNK],
                                lhsT=kt_sb[hpass][h2 * DH:(h2 + 1) * DH,
                                                  jt * P:(jt + 1) * P],
                                rhs=qt_sb[hpass][h2 * DH:(h2 + 1) * DH,
                                                 c0 + rel:c0 + CHUNK],
                                start=True, stop=True,
                                tile_position=(h2 * DH, 0))
                        # one exp for both heads (both psum banks)
                        nc.scalar.activation(
                            es.rearrange("p (t c) -> p t c", t=2)[:, :, rel:],
                            s2.rearrange("p (t c) -> p t c", t=2)[:, :, rel:],
                            ActFn.Exp)
                        if diag:
                            for h2 in range(2):
                                nc.vector.tensor_mul(
                                    es[:, h2 * CHUNK + rel:
                                       h2 * CHUNK + rel + P],
                                    es[:, h2 * CHUNK + rel:
                                       h2 * CHUNK + rel + P],
                                    mask_sb[:])
                        for h2 in range(2):
                            h = 2 * hpass + h2
                            # O^T(+sums) accumulation: V_aug^T @ expS^T
                            nc.tensor.matmul(
                                ot_ps[h2][:, rel:CHUNK],
                                lhsT=v_sb[jt][:, h * DHA:(h + 1) * DHA],
                                rhs=es[:, h2 * CHUNK + rel:(h2 + 1) * CHUNK],
                                start=(jt == 0), stop=(jt == jt_end - 1))
                        pop_work(late_ok=(jt >= jt_end - 2))

                    # evacuate psum fast, normalize off the PE critical path
                    for h2 in range(2):
                        h = 2 * hpass + h2
                        otr = sb.tile([DH, CHUNK], F32, tag=f"otr{h2}",
                                      name=f"otr{ci}_{h}", bufs=2)
                        srow = sb.tile([1, CHUNK], F32, tag=f"sr{h2}",
                                       name=f"sr{ci}_{h}", bufs=2)
                        nc.vector.tensor_copy(otr[:], ot_ps[h2][0:DH, :])
                        nc.vector.tensor_copy(srow[:], ot_ps[h2][DH:DHA, :])
                        rcp = sb.tile([1, CHUNK], F32, tag=f"rcp{h2}",
                                      name=f"rcp{ci}_{h}", bufs=2)
                        nc.vector.reciprocal_approx_fast(rcp[:], srow[:])
                        bc_sb = sb.tile([DH, CHUNK], F32, tag=f"bc{h2}",
                                        name=f"bc{ci}_{h}", bufs=2)
                        nc.gpsimd.partition_broadcast(bc_sb[:], rcp[:],
                                                      channels=DH)
                        nc.vector.tensor_mul(
                            ot_sb[hpass][h2 * DH:(h2 + 1) * DH,
                                         c0:c0 + CHUNK],
                            otr[:], bc_sb[:])

                    # this pair's half of the chunk goes out now
                    last_parts.update(emit_ag_pair(ci, hpass))

                agt = dict(last_parts)
                last_parts = {}
                slices = [(agt[k], 0) for k in range(nko)]
                evens = [k for k in range(nko) if k % 2 == 0]
                odds = [k for k in range(nko) if k % 2 == 1]
                nm = HS // P
                if not last:
                    for m in range(nm):
                        work_late.append(
                            lambda ci=ci, m=m, s=slices, ko=evens + odds:
                            emit_proj(ci, m, s, ko))
                else:
                    # split each output block's projection: the even k-tiles
                    # (from the pair-A AllGather) run while pair-B flies
                    op_tiles = {}

                    def proj_ev(ci, m, s):
                        op_tiles[m] = emit_proj(ci, m, s, evens, evac=False)

                    def proj_od(ci, m, s):
                        emit_proj(ci, m, s, odds, op_ps=op_tiles.pop(m))

                    for m in range(nm):
                        work_late.append(
                            lambda ci=ci, m=m, s=slices: proj_ev(ci, m, s))
                    for m in range(nm):
                        work_late.append(
                            lambda ci=ci, m=m, s=slices: proj_od(ci, m, s))

            while work_early or work_late:
                pop_work(late_ok=True)

    if compile:
        nc.compile()
    return nc


def make_in_maps(x, Wq, Wk, Wv, Wo, n_cores=N_CORES):
    import ml_dtypes
    bf16 = ml_dtypes.bfloat16
    scale = np.float32(DH ** -0.5)
    # band mask for the diagonal j-tile of S^T [j,i]: keep j <= i
    mask_b = np.triu(np.ones((P, P), np.float32)).astype(bf16)
    in_maps = []
    for c in range(n_cores):
        b, r = divmod(c, 4)
        hs = r * HS
        in_maps.append({
            "xT": np.ascontiguousarray(x[b].T).astype(bf16),
            "wq": (Wq[:, hs:hs + HS] * scale).astype(bf16),
            "wk": np.ascontiguousarray(Wk[:, hs:hs + HS]).astype(bf16),
            "wv": np.ascontiguousarray(Wv[:, hs:hs + HS]).astype(bf16),
            "wo": np.ascontiguousarray(Wo[:, hs:hs + HS]).astype(bf16),
            "mask_c": mask_b,
        })
    return in_maps


def assemble_out(results, B, seq, n_cores=N_CORES):
    out = np.empty((B, seq, INNER), np.float32)
    for c in range(n_cores):
        b, r = divmod(c, 4)
        out[b][:, r * HS:(r + 1) * HS] = results[c]["outT"].T
    return out


_NC_CACHE = {}


def kernel(x, Wq, Wk, Wv, Wo):
    from concourse import bass_utils
    x = np.asarray(x, np.float32)
    B, seq, dim = x.shape
    key = (seq, dim)
    if key not in _NC_CACHE:
        _NC_CACHE[key] = build_nc(seq=seq, dim=dim)
    nc = _NC_CACHE[key]
    in_maps = make_in_maps(x, np.asarray(Wq, np.float32),
                           np.asarray(Wk, np.float32),
                           np.asarray(Wv, np.float32),
                           np.asarray(Wo, np.float32))
    res = bass_utils.run_bass_kernel_spmd(
        nc, in_maps, core_ids=list(range(N_CORES)))
    return assemble_out(res.results, B, seq)


# revision 18
# speedup vs baseline: 1.0496x; 1.0496x over previous
"""Distributed causal multi-head attention for one TRN2 chip (8 NeuronCores).

Sharding: batch (2) x head-groups (4 heads/core) -> 8 cores.
Core c handles batch c//4, heads [ (c%4)*4 , (c%4)*4+4 ).
Per core: QKV projections for its 4 heads, flash-style causal attention
with scores kept transposed (S^T = K @ Q^T) so the PV product needs no
transposes; V is augmented with a ones column so the softmax denominators
fall out of the same matmul (row 64 of each head's O^T psum).  Then an
AllGather of the attention output (pre-Wo, 4-core group = one batch) and
a column-sliced output projection.  Host assembles the 8 column/batch
shards.  Compute dtype bf16 (PSUM accumulation fp32), softmax in fp32.

Scheduling: the attention loop keeps the in-order PE queue saturated by
interleaving KT/V projections for later chunks and the AllGather-gated
output projections as queued work items (so the HAM clock gate never
re-throttles).  Chunk 0+1 share one AllGather (it completes under chunk
2's attention); the last chunk's AllGather is split per head-pair so the
first half overlaps pass B and the output projection can start on half
the k-tiles while the second half is still in flight.
"""

import sys
from collections import deque

import numpy as np

sys.path.insert(0, "/opt/trn_rl_repo")

import concourse.bass as bass  # noqa: E402
import concourse.bacc as bacc  # noqa: E402
import concourse.tile as tile  # noqa: E402
import concourse.mybir as mybir  # noqa: E402

F32 = mybir.dt.float32
BF16 = mybir.dt.bfloat16
ActFn = mybir.ActivationFunctionType

P = 128          # partition dim
CHUNK = 512      # i-chunk (matmul moving free dim, one psum bank of fp32)
DH = 64          # head dim
HPC = 4          # heads per core
HS = HPC * DH    # 256 per-core inner slice
DHA = DH + 1     # augmented head dim (ones column for softmax sums)
INNER = 1024     # total inner dim (16 heads x 64)
N_CORES = 8
GROUPS = [[0, 1, 2, 3], [4, 5, 6, 7]]


def build_nc(seq=2048, dim=1024, n_cores=N_CORES, groups=GROUPS, compile=True):
    """Build the SPMD Bass graph (identical on all cores)."""
    nch = seq // CHUNK          # i-chunks
    jpc = CHUNK // P            # j-tiles per chunk (4)
    njt = seq // P              # j-tiles
    nk = dim // P               # feature k-tiles
    nko = INNER // P            # inner k-tiles for the output projection
    grp = len(groups[0])        # replica group size (4)

    nc = bacc.Bacc("TRN2", target_bir_lowering=False, debug=False,
                   enable_asserts=False, num_devices=n_cores)

    xT = nc.dram_tensor("xT", [dim, seq], BF16, kind="ExternalInput").ap()
    wq = nc.dram_tensor("wq", [dim, HS], BF16, kind="ExternalInput").ap()
    wk = nc.dram_tensor("wk", [dim, HS], BF16, kind="ExternalInput").ap()
    wv = nc.dram_tensor("wv", [dim, HS], BF16, kind="ExternalInput").ap()
    wo = nc.dram_tensor("wo", [INNER, HS], BF16, kind="ExternalInput").ap()
    mask_c = nc.dram_tensor("mask_c", [P, P], BF16, kind="ExternalInput").ap()
    outT = nc.dram_tensor("outT", [HS, seq], F32, kind="ExternalOutput").ap()

    with tile.TileContext(nc) as tc:
        with tc.tile_pool(name="sb", bufs=1) as sb, \
             tc.tile_pool(name="ps", bufs=1, space="PSUM") as ps, \
             tc.tile_pool(name="dram", bufs=1, space="DRAM") as dram:

            # ---- load inputs ----
            xt = [sb.tile([P, seq], BF16, tag=f"xt{k}", name=f"xt{k}")
                  for k in range(nk)]
            wq_sb = [sb.tile([P, HS], BF16, tag=f"wq{k}", name=f"wq{k}")
                     for k in range(nk)]
            wk_sb = [sb.tile([P, HS], BF16, tag=f"wk{k}", name=f"wk{k}")
                     for k in range(nk)]
            wv_sb = [sb.tile([P, HS], BF16, tag=f"wv{k}", name=f"wv{k}")
                     for k in range(nk)]
            wo_sb = [sb.tile([P, HS], BF16, tag=f"wo{k}", name=f"wo{k}")
                     for k in range(nko)]
            mask_sb = sb.tile([P, P], BF16, tag="mask", name="mask")

            # inputs spread across engine DMA queues so the loads issue
            # in parallel (the sync queue alone serializes ~40 descriptors)
            for k in range(nk):
                nc.sync.dma_start(xt[k][:], xT[k * P:(k + 1) * P, :])
                nc.scalar.dma_start(wq_sb[k][:], wq[k * P:(k + 1) * P, :])
                nc.scalar.dma_start(wk_sb[k][:], wk[k * P:(k + 1) * P, :])
                nc.gpsimd.dma_start(wv_sb[k][:], wv[k * P:(k + 1) * P, :])
            for k in range(nko):
                nc.gpsimd.dma_start(wo_sb[k][:], wo[k * P:(k + 1) * P, :])
            nc.gpsimd.dma_start(mask_sb[:], mask_c[:])

            # warm up the collectives firmware while QKV runs
            warm_in = dram.tile([P, 4], BF16, tag="warm_i", name="warm_i")
            warm_out = dram.tile([grp * P, 4], BF16,
                                 tag="warm_o", name="warm_o")
            nc.sync.dma_start(warm_in[:], mask_c[0:P, 0:4])
            nc.gpsimd.collective_compute(
                "AllGather", mybir.AluOpType.bypass, replica_groups=groups,
                ins=[warm_in.opt()], outs=[warm_out.opt()])

            # persistent QKV results
            qt_sb = [sb.tile([P, seq], BF16, tag=f"qt{p}", name=f"qt{p}")
                     for p in range(2)]
            kt_sb = [sb.tile([P, seq], BF16, tag=f"kt{p}", name=f"kt{p}")
                     for p in range(2)]
            v_sb = [sb.tile([P, HPC * DHA], BF16, tag=f"v{j}", name=f"v{j}")
                    for j in range(njt)]
            ot_sb = [sb.tile([P, seq], BF16, tag=f"ot{p}", name=f"ot{p}")
                     for p in range(2)]

            # ---- interleavable work items (each emits one psum group) ----
            def emit_kt(pair, ch):
                pt = ps.tile([P, CHUNK], F32, tag="misc",
                             name=f"ktps{pair}_{ch}", bufs=2)
                for k in range(nk):
                    nc.tensor.matmul(
                        pt[:], lhsT=wk_sb[k][:, pair * P:(pair + 1) * P],
                        rhs=xt[k][:, ch * CHUNK:(ch + 1) * CHUNK],
                        start=(k == 0), stop=(k == nk - 1))
                nc.scalar.activation(
                    kt_sb[pair][:, ch * CHUNK:(ch + 1) * CHUNK], pt[:],
                    ActFn.Copy)

            def emit_v(jt):
                pt = ps.tile([P, HS], F32, tag="misc",
                             name=f"vps{jt}", bufs=2)
                for k in range(nk):
                    nc.tensor.matmul(
                        pt[:], lhsT=xt[k][:, jt * P:(jt + 1) * P],
                        rhs=wv_sb[k][:],
                        start=(k == 0), stop=(k == nk - 1))
                nc.scalar.activation(
                    v_sb[jt].rearrange("p (h d) -> p h d", h=HPC)[:, :, 0:DH],
                    pt.rearrange("p (h d) -> p h d", h=HPC), ActFn.Copy)
                nc.vector.memset(
                    v_sb[jt].rearrange("p (h d) -> p h d", h=HPC)[:, :, DH:DHA],
                    1.0)

            def emit_proj(ci, m, slices, korder, op_ps=None, evac=True):
                # transposed output block: outT[m*128:(m+1)*128, chunk ci]
                # = Wo[:, m-slice].T @ attT[:, chunk] over the k-tiles in
                # `korder` (a partial pass keeps op_ps alive).  The moving
                # operand is the gathered attT tile (N=512, half the
                # matmuls of the untransposed form); the host untransposes.
                c0 = ci * CHUNK
                first = op_ps is None
                if first:
                    op_ps = ps.tile([P, CHUNK], F32, tag="misc",
                                    name=f"op{ci}_{m}", bufs=2)
                for n, k in enumerate(korder):
                    ag_t, coff = slices[k]
                    nc.tensor.matmul(
                        op_ps[:],
                        lhsT=wo_sb[k][:, m * P:(m + 1) * P],
                        rhs=ag_t[:, coff:coff + CHUNK],
                        start=(first and n == 0),
                        stop=(evac and n == len(korder) - 1))
                if not evac:
                    return op_ps
                o_sb = sb.tile([P, CHUNK], F32, tag="osb",
                               name=f"o{ci}_{m}", bufs=2)
                nc.vector.tensor_copy(o_sb[:], op_ps[:])
                nc.sync.dma_start(
                    outT[m * P:(m + 1) * P, c0:c0 + CHUNK], o_sb[:])
                return None

            work_early = deque()   # KT/V for future chunks (not gated)
            work_late = deque()    # output projections (gated on AllGather)

            def pop_work(late_ok):
                if work_early:
                    work_early.popleft()()
                    if len(work_early) > 4:
                        work_early.popleft()()
                elif late_ok and work_late:
                    work_late.popleft()()

            def emit_ag_pair(ci, pair):
                # half AllGather (one head pair) of the chunk `ci` — fired
                # right after that pair's normalize, so pair A overlaps the
                # second attention pass and both stay small (cheap on CC).
                # The gathered k-tiles (2r+pair) load right behind it.
                c0 = ci * CHUNK
                bounce_in = dram.tile([P, CHUNK], BF16, tag=f"binh{pair}",
                                      name=f"binh{ci}_{pair}", bufs=2)
                bounce_out = dram.tile([grp * P, CHUNK], BF16,
                                       tag=f"bouth{pair}",
                                       name=f"bouth{ci}_{pair}", bufs=2)
                nc.sync.dma_start(bounce_in[:], ot_sb[pair][:, c0:c0 + CHUNK])
                nc.gpsimd.collective_compute(
                    "AllGather", mybir.AluOpType.bypass,
                    replica_groups=groups,
                    ins=[bounce_in.opt()], outs=[bounce_out.opt()])
                tiles = {}
                for r in range(grp):
                    k = 2 * r + pair
                    t = sb.tile([P, CHUNK], BF16, tag=f"ag{k}",
                                name=f"ag{ci}_{k}", bufs=2)
                    nc.sync.dma_start(t[:], bounce_out[r * P:(r + 1) * P, :])
                    tiles[k] = t
                return tiles

            # ---- upfront projections: chunk-0/1 Q, chunk-0 K, chunk-0 V
            # (Q for later chunks is deferred into the work queue).  Q is
            # k-outer / weight-stationary so it pipelines with the xT DMAs;
            # two chunks share one 2-bank psum tile.
            def emit_qt(pair, chh):
                wch = min(2, nch - chh)
                pt = ps.tile([P, wch * CHUNK], F32, tag="s2",
                             name=f"qps{pair}_{chh}", bufs=2)
                for k in range(nk):
                    for c in range(wch):
                        nc.tensor.matmul(
                            pt[:, c * CHUNK:(c + 1) * CHUNK],
                            lhsT=wq_sb[k][:, pair * P:(pair + 1) * P],
                            rhs=xt[k][:, (chh + c) * CHUNK:
                                       (chh + c + 1) * CHUNK],
                            start=(k == 0), stop=(k == nk - 1))
                nc.scalar.activation(
                    qt_sb[pair][:, chh * CHUNK:(chh + wch) * CHUNK],
                    pt[:], ActFn.Copy)

            for pair in range(2):
                emit_qt(pair, 0)
            for pair in range(2):
                emit_kt(pair, 0)
            for jt in range(jpc):
                emit_v(jt)
            for chh in range(2, nch, 2):
                for pair in range(2):
                    work_early.append(
                        lambda pair=pair, chh=chh: emit_qt(pair, chh))

            # ---- attention chunks ----
            last_parts = {}
            for ci in range(nch):
                jt_end = jpc * (ci + 1)
                c0 = ci * CHUNK
                last = ci == nch - 1

                if ci + 1 < nch:
                    for pair in range(2):
                        work_early.append(
                            lambda pair=pair, ch=ci + 1: emit_kt(pair, ch))
                    for jt in range(jpc * (ci + 1), jpc * (ci + 2)):
                        work_early.append(lambda jt=jt: emit_v(jt))

                for hpass in range(2):
                    # heads 2*hpass, 2*hpass+1  (== head pair `hpass`)
                    ot_ps = [ps.tile([DHA, CHUNK], F32, tag=f"ot{h2}",
                                     name=f"ot{ci}_{hpass}_{h2}", bufs=1)
                             for h2 in range(2)]
                    for jt in range(jt_end):
                        rel = max(0, (jt - jpc * ci)) * P
                        diag = jt >= jpc * ci

                        s2 = ps.tile([P, 2 * CHUNK], F32, tag="s2",
                                     name=f"s{ci}_{hpass}_{jt}", bufs=2)
                        es = sb.tile([P, 2 * CHUNK], BF16, tag="es",
                                     name=f"es{ci}_{hpass}_{jt}", bufs=3)

                        for h2 in range(2):
                            # S^T tile = K_h @ Q_h^T (row-tiled, K=64)
                            nc.tensor.matmul(
                                s2[:, h2 * CHUNK + rel:(h2 + 1) * CHUNK],
                                lhsT=kt_sb[hpass][h2 * DH:(h2 + 1) * DH,
                                                  jt * P:(jt + 1) * P],
                                rhs=qt_sb[hpass][h2 * DH:(h2 + 1) * DH,
                                                 c0 + rel:c0 + CHUNK],
                                start=True, stop=True,
                                tile_position=(h2 * DH, 0))
                        # one exp for both heads (both psum banks)
                        nc.scalar.activation(
                            es.rearrange("p (t c) -> p t c", t=2)[:, :, rel:],
                            s2.rearrange("p (t c) -> p t c", t=2)[:, :, rel:],
                            ActFn.Exp)
                        if diag:
                            for h2 in range(2):
                                nc.vector.tensor_mul(
                                    es[:, h2 * CHUNK + rel:
                                       h2 * CHUNK + rel + P],
                                    es[:, h2 * CHUNK + rel:
                                       h2 * CHUNK + rel + P],
                                    mask_sb[:])
                        for h2 in range(2):
                            h = 2 * hpass + h2
                            # O^T(+sums) accumulation: V_aug^T @ expS^T
                            nc.tensor.matmul(
                                ot_ps[h2][:, rel:CHUNK],
                                lhsT=v_sb[jt][:, h * DHA:(h + 1) * DHA],
                                rhs=es[:, h2 * CHUNK + rel:(h2 + 1) * CHUNK],
                                start=(jt == 0), stop=(jt == jt_end - 1))
                        pop_work(late_ok=(jt >= jt_end - 2))

                    # evacuate psum fast, normalize off the PE critical path
                    for h2 in range(2):
                        h = 2 * hpass + h2
                        otr = sb.tile([DH, CHUNK], F32, tag=f"otr{h2}",
                                      name=f"otr{ci}_{h}", bufs=2)
                        srow = sb.tile([1, CHUNK], F32, tag=f"sr{h2}",
                                       name=f"sr{ci}_{h}", bufs=2)
                        nc.vector.tensor_copy(otr[:], ot_ps[h2][0:DH, :])
                        nc.vector.tensor_copy(srow[:], ot_ps[h2][DH:DHA, :])
                        rcp = sb.tile([1, CHUNK], F32, tag=f"rcp{h2}",
                                      name=f"rcp{ci}_{h}", bufs=2)
                        nc.vector.reciprocal_approx_fast(rcp[:], srow[:])
                        bc_sb = sb.tile([DH, CHUNK], F32, tag=f"bc{h2}",
                                        name=f"bc{ci}_{h}", bufs=2)
                        nc.gpsimd.partition_broadcast(bc_sb[:], rcp[:],
                                                      channels=DH)
                        nc.vector.tensor_mul(
                            ot_sb[hpass][h2 * DH:(h2 + 1) * DH,
                                         c0:c0 + CHUNK],
                            otr[:], bc_sb[:])

                    # this pair's half of the chunk goes out now
                    last_parts.update(emit_ag_pair(ci, hpass))

                agt = dict(last_parts)
                last_parts = {}
                slices = [(agt[k], 0) for k in range(nko)]
                evens = [k for k in range(nko) if k % 2 == 0]
                odds = [k for k in range(nko) if k % 2 == 1]
                nm = HS // P
                if not last:
                    for m in range(nm):
                        work_late.append(
                            lambda ci=ci, m=m, s=slices, ko=evens + odds:
                            emit_proj(ci, m, s, ko))
                else:
                    # split each output block's projection: the even k-tiles
                    # (from the pair-A AllGather) run while pair-B flies
                    op_tiles = {}

                    def proj_ev(ci, m, s):
                        op_tiles[m] = emit_proj(ci, m, s, evens, evac=False)

                    def proj_od(ci, m, s):
                        emit_proj(ci, m, s, odds, op_ps=op_tiles.pop(m))

                    for m in range(nm):
                        work_late.append(
                            lambda ci=ci, m=m, s=slices: proj_ev(ci, m, s))
                    for m in range(nm):
                        work_late.append(
                            lambda ci=ci, m=m, s=slices: proj_od(ci, m, s))

            while work_early or work_late:
                pop_work(late_ok=True)

    if compile:
        nc.compile()
    return nc


def make_in_maps(x, Wq, Wk, Wv, Wo, n_cores=N_CORES):
    import ml_dtypes
    bf16 = ml_dtypes.bfloat16
    scale = np.float32(DH ** -0.5)
    # band mask for the diagonal j-tile of S^T [j,i]: keep j <= i
    mask_b = np.triu(np.ones((P, P), np.float32)).astype(bf16)
    in_maps = []
    for c in range(n_cores):
        b, r = divmod(c, 4)
        hs = r * HS
        in_maps.append({
            "xT": np.ascontiguousarray(x[b].T).astype(bf16),
            "wq": (Wq[:, hs:hs + HS] * scale).astype(bf16),
            "wk": np.ascontiguousarray(Wk[:, hs:hs + HS]).astype(bf16),
            "wv": np.ascontiguousarray(Wv[:, hs:hs + HS]).astype(bf16),
            "wo": np.ascontiguousarray(Wo[:, hs:hs + HS]).astype(bf16),
            "mask_c": mask_b,
        })
    return in_maps


def assemble_out(results, B, seq, n_cores=N_CORES):
    out = np.empty((B, seq, INNER), np.float32)
    for c in range(n_cores):
        b, r = divmod(c, 4)
        out[b][:, r * HS:(r + 1) * HS] = results[c]["outT"].T
    return out


_NC_CACHE = {}


def kernel(x, Wq, Wk, Wv, Wo):
    from concourse import bass_utils
    x = np.asarray(x, np.float32)
    B, seq, dim = x.shape
    key = (seq, dim)
    if key not in _NC_CACHE:
        _NC_CACHE[key] = build_nc(seq=seq, dim=dim)
    nc = _NC_CACHE[key]
    in_maps = make_in_maps(x, np.asarray(Wq, np.float32),
                           np.asarray(Wk, np.float32),
                           np.asarray(Wv, np.float32),
                           np.asarray(Wo, np.float32))
    res = bass_utils.run_bass_kernel_spmd(
        nc, in_maps, core_ids=list(range(N_CORES)))
    return assemble_out(res.results, B, seq)


# revision 19
# speedup vs baseline: 1.0982x; 1.0462x over previous
"""Distributed causal multi-head attention for one TRN2 chip (8 NeuronCores).

Sharding: batch (2) x head-groups (4 heads/core) -> 8 cores.
Core c handles batch c//4, heads [ (c%4)*4 , (c%4)*4+4 ).
Per core: QKV projections for its 4 heads, flash-style causal attention
with scores kept transposed (S^T = K @ Q^T) so the PV product needs no
transposes; V is augmented with a ones column so the softmax denominators
fall out of the same matmul (row 64 of each head's O^T psum).  Then an
AllGather of the attention output (pre-Wo, 4-core group = one batch) and
a column-sliced output projection.  Host assembles the 8 column/batch
shards.  Compute dtype bf16 (PSUM accumulation fp32), softmax in fp32.

Scheduling: the attention loop keeps the in-order PE queue saturated by
interleaving KT/V projections for later chunks and the AllGather-gated
output projections as queued work items (so the HAM clock gate never
re-throttles).  Chunk 0+1 share one AllGather (it completes under chunk
2's attention); the last chunk's AllGather is split per head-pair so the
first half overlaps pass B and the output projection can start on half
the k-tiles while the second half is still in flight.
"""

import sys
from collections import deque

import numpy as np

sys.path.insert(0, "/opt/trn_rl_repo")

import concourse.bass as bass  # noqa: E402
import concourse.bacc as bacc  # noqa: E402
import concourse.tile as tile  # noqa: E402
import concourse.mybir as mybir  # noqa: E402

F32 = mybir.dt.float32
BF16 = mybir.dt.bfloat16
ActFn = mybir.ActivationFunctionType

P = 128          # partition dim
CHUNK = 512      # i-chunk (matmul moving free dim, one psum bank of fp32)
DH = 64          # head dim
HPC = 4          # heads per core
HS = HPC * DH    # 256 per-core inner slice
DHA = DH + 1     # augmented head dim (ones column for softmax sums)
INNER = 1024     # total inner dim (16 heads x 64)
N_CORES = 8
GROUPS = [[0, 1, 2, 3], [4, 5, 6, 7]]


def build_nc(seq=2048, dim=1024, n_cores=N_CORES, groups=GROUPS, compile=True):
    """Build the SPMD Bass graph (identical on all cores)."""
    nch = seq // CHUNK          # i-chunks
    jpc = CHUNK // P            # j-tiles per chunk (4)
    njt = seq // P              # j-tiles
    nk = dim // P               # feature k-tiles
    nko = INNER // P            # inner k-tiles for the output projection
    grp = len(groups[0])        # replica group size (4)

    nc = bacc.Bacc("TRN2", target_bir_lowering=False, debug=False,
                   enable_asserts=False, num_devices=n_cores)

    xT = nc.dram_tensor("xT", [dim, seq], BF16, kind="ExternalInput").ap()
    wq = nc.dram_tensor("wq", [dim, HS], BF16, kind="ExternalInput").ap()
    wk = nc.dram_tensor("wk", [dim, HS], BF16, kind="ExternalInput").ap()
    wv = nc.dram_tensor("wv", [dim, HS], BF16, kind="ExternalInput").ap()
    wo = nc.dram_tensor("wo", [INNER, HS], BF16, kind="ExternalInput").ap()
    mask_c = nc.dram_tensor("mask_c", [P, P], BF16, kind="ExternalInput").ap()
    outT = nc.dram_tensor("outT", [HS, seq], F32, kind="ExternalOutput").ap()

    with tile.TileContext(nc) as tc:
        with tc.tile_pool(name="sb", bufs=1) as sb, \
             tc.tile_pool(name="ps", bufs=1, space="PSUM") as ps, \
             tc.tile_pool(name="dram", bufs=1, space="DRAM") as dram:

            # ---- load inputs ----
            xt = [sb.tile([P, seq], BF16, tag=f"xt{k}", name=f"xt{k}")
                  for k in range(nk)]
            wq_sb = [sb.tile([P, HS], BF16, tag=f"wq{k}", name=f"wq{k}")
                     for k in range(nk)]
            wk_sb = [sb.tile([P, HS], BF16, tag=f"wk{k}", name=f"wk{k}")
                     for k in range(nk)]
            wv_sb = [sb.tile([P, HS], BF16, tag=f"wv{k}", name=f"wv{k}")
                     for k in range(nk)]
            wo_sb = [sb.tile([P, HS], BF16, tag=f"wo{k}", name=f"wo{k}")
                     for k in range(nko)]
            mask_sb = sb.tile([P, P], BF16, tag="mask", name="mask")

            # inputs spread across engine DMA queues so the loads issue
            # in parallel (the sync queue alone serializes ~40 descriptors)
            for k in range(nk):
                nc.sync.dma_start(xt[k][:], xT[k * P:(k + 1) * P, :])
                nc.scalar.dma_start(wq_sb[k][:], wq[k * P:(k + 1) * P, :])
                nc.scalar.dma_start(wk_sb[k][:], wk[k * P:(k + 1) * P, :])
                nc.gpsimd.dma_start(wv_sb[k][:], wv[k * P:(k + 1) * P, :])
            for k in range(nko):
                nc.gpsimd.dma_start(wo_sb[k][:], wo[k * P:(k + 1) * P, :])
            nc.gpsimd.dma_start(mask_sb[:], mask_c[:])

            # warm up the collectives firmware while QKV runs
            warm_in = dram.tile([P, 4], BF16, tag="warm_i", name="warm_i")
            warm_out = dram.tile([grp * P, 4], BF16,
                                 tag="warm_o", name="warm_o")
            nc.sync.dma_start(warm_in[:], mask_c[0:P, 0:4])
            nc.gpsimd.collective_compute(
                "AllGather", mybir.AluOpType.bypass, replica_groups=groups,
                ins=[warm_in.opt()], outs=[warm_out.opt()])

            # persistent QKV results
            qt_sb = [sb.tile([P, seq], BF16, tag=f"qt{p}", name=f"qt{p}")
                     for p in range(2)]
            kt_sb = [sb.tile([P, seq], BF16, tag=f"kt{p}", name=f"kt{p}")
                     for p in range(2)]
            v_sb = [sb.tile([P, HPC * DHA], BF16, tag=f"v{j}", name=f"v{j}")
                    for j in range(njt)]
            ot_sb = [sb.tile([P, seq], BF16, tag=f"ot{p}", name=f"ot{p}")
                     for p in range(2)]

            # ---- interleavable work items (each emits one psum group) ----
            def emit_kt(pair, ch):
                pt = ps.tile([P, CHUNK], F32, tag="misc",
                             name=f"ktps{pair}_{ch}", bufs=2)
                for k in range(nk):
                    nc.tensor.matmul(
                        pt[:], lhsT=wk_sb[k][:, pair * P:(pair + 1) * P],
                        rhs=xt[k][:, ch * CHUNK:(ch + 1) * CHUNK],
                        start=(k == 0), stop=(k == nk - 1))
                nc.scalar.activation(
                    kt_sb[pair][:, ch * CHUNK:(ch + 1) * CHUNK], pt[:],
                    ActFn.Copy)

            def emit_v(jt):
                pt = ps.tile([P, HS], F32, tag="misc",
                             name=f"vps{jt}", bufs=2)
                for k in range(nk):
                    nc.tensor.matmul(
                        pt[:], lhsT=xt[k][:, jt * P:(jt + 1) * P],
                        rhs=wv_sb[k][:],
                        start=(k == 0), stop=(k == nk - 1))
                nc.scalar.activation(
                    v_sb[jt].rearrange("p (h d) -> p h d", h=HPC)[:, :, 0:DH],
                    pt.rearrange("p (h d) -> p h d", h=HPC), ActFn.Copy)
                nc.vector.memset(
                    v_sb[jt].rearrange("p (h d) -> p h d", h=HPC)[:, :, DH:DHA],
                    1.0)

            def emit_proj(ci, m, slices, korder, op_ps=None, evac=True):
                # transposed output block: outT[m*128:(m+1)*128, chunk ci]
                # = Wo[:, m-slice].T @ attT[:, chunk] over the k-tiles in
                # `korder` (a partial pass keeps op_ps alive).  The moving
                # operand is the gathered attT tile (N=512, half the
                # matmuls of the untransposed form); the host untransposes.
                c0 = ci * CHUNK
                first = op_ps is None
                if first:
                    op_ps = ps.tile([P, CHUNK], F32, tag="misc",
                                    name=f"op{ci}_{m}", bufs=2)
                for n, k in enumerate(korder):
                    ag_t, coff = slices[k]
                    nc.tensor.matmul(
                        op_ps[:],
                        lhsT=wo_sb[k][:, m * P:(m + 1) * P],
                        rhs=ag_t[:, coff:coff + CHUNK],
                        start=(first and n == 0),
                        stop=(evac and n == len(korder) - 1))
                if not evac:
                    return op_ps
                o_sb = sb.tile([P, CHUNK], F32, tag="osb",
                               name=f"o{ci}_{m}", bufs=2)
                nc.vector.tensor_copy(o_sb[:], op_ps[:])
                nc.sync.dma_start(
                    outT[m * P:(m + 1) * P, c0:c0 + CHUNK], o_sb[:])
                return None

            work_early = deque()   # KT/V for future chunks (not gated)
            work_late = deque()    # output projections (gated on AllGather)

            def pop_work(late_ok):
                if work_early:
                    work_early.popleft()()
                    if len(work_early) > 4:
                        work_early.popleft()()
                elif late_ok and work_late:
                    work_late.popleft()()

            def emit_ag_full(ci, bounce_in):
                # one AllGather for both head pairs of chunk `ci` (256KB —
                # amortizes the ncfw floor; rank-major rows land so that
                # gathered row-block k*128 is exactly attT k-tile k)
                bounce_out = dram.tile([grp * 2 * P, CHUNK], BF16,
                                       tag="boutf", name=f"boutf{ci}", bufs=2)
                nc.gpsimd.collective_compute(
                    "AllGather", mybir.AluOpType.bypass,
                    replica_groups=groups,
                    ins=[bounce_in.opt()], outs=[bounce_out.opt()])
                tiles = {}
                for k in range(nko):
                    t = sb.tile([P, CHUNK], BF16, tag=f"ag{k}",
                                name=f"ag{ci}_{k}", bufs=2)
                    nc.sync.dma_start(t[:],
                                      bounce_out[k * P:(k + 1) * P, :])
                    tiles[k] = t
                return tiles

            def emit_ag_pair(ci, pair):
                # half AllGather (one head pair) of the chunk `ci` — fired
                # right after that pair's normalize, so pair A overlaps the
                # second attention pass and both stay small (cheap on CC).
                # The gathered k-tiles (2r+pair) load right behind it.
                c0 = ci * CHUNK
                bounce_in = dram.tile([P, CHUNK], BF16, tag=f"binh{pair}",
                                      name=f"binh{ci}_{pair}", bufs=2)
                bounce_out = dram.tile([grp * P, CHUNK], BF16,
                                       tag=f"bouth{pair}",
                                       name=f"bouth{ci}_{pair}", bufs=2)
                nc.sync.dma_start(bounce_in[:], ot_sb[pair][:, c0:c0 + CHUNK])
                nc.gpsimd.collective_compute(
                    "AllGather", mybir.AluOpType.bypass,
                    replica_groups=groups,
                    ins=[bounce_in.opt()], outs=[bounce_out.opt()])
                tiles = {}
                for r in range(grp):
                    k = 2 * r + pair
                    t = sb.tile([P, CHUNK], BF16, tag=f"ag{k}",
                                name=f"ag{ci}_{k}", bufs=2)
                    nc.sync.dma_start(t[:], bounce_out[r * P:(r + 1) * P, :])
                    tiles[k] = t
                return tiles

            # ---- upfront projections: chunk-0/1 Q, chunk-0 K, chunk-0 V
            # (Q for later chunks is deferred into the work queue).  Q is
            # k-outer / weight-stationary so it pipelines with the xT DMAs;
            # two chunks share one 2-bank psum tile.
            def emit_qt(pair, chh):
                wch = min(2, nch - chh)
                pt = ps.tile([P, wch * CHUNK], F32, tag="s2",
                             name=f"qps{pair}_{chh}", bufs=2)
                for k in range(nk):
                    for c in range(wch):
                        nc.tensor.matmul(
                            pt[:, c * CHUNK:(c + 1) * CHUNK],
                            lhsT=wq_sb[k][:, pair * P:(pair + 1) * P],
                            rhs=xt[k][:, (chh + c) * CHUNK:
                                       (chh + c + 1) * CHUNK],
                            start=(k == 0), stop=(k == nk - 1))
                nc.scalar.activation(
                    qt_sb[pair][:, chh * CHUNK:(chh + wch) * CHUNK],
                    pt[:], ActFn.Copy)

            for pair in range(2):
                emit_qt(pair, 0)
            for pair in range(2):
                emit_kt(pair, 0)
            for jt in range(jpc):
                emit_v(jt)
            for chh in range(2, nch, 2):
                for pair in range(2):
                    work_early.append(
                        lambda pair=pair, chh=chh: emit_qt(pair, chh))

            # ---- attention chunks ----
            last_parts = {}
            for ci in range(nch):
                jt_end = jpc * (ci + 1)
                c0 = ci * CHUNK
                last = ci == nch - 1

                if ci + 1 < nch:
                    for pair in range(2):
                        work_early.append(
                            lambda pair=pair, ch=ci + 1: emit_kt(pair, ch))
                    for jt in range(jpc * (ci + 1), jpc * (ci + 2)):
                        work_early.append(lambda jt=jt: emit_v(jt))

                binf = None if last else dram.tile(
                    [2 * P, CHUNK], BF16, tag="binf", name=f"binf{ci}",
                    bufs=2)

                for hpass in range(2):
                    # heads 2*hpass, 2*hpass+1  (== head pair `hpass`)
                    ot_ps = [ps.tile([DHA, CHUNK], F32, tag=f"ot{h2}",
                                     name=f"ot{ci}_{hpass}_{h2}", bufs=1)
                             for h2 in range(2)]
                    for jt in range(jt_end):
                        rel = max(0, (jt - jpc * ci)) * P
                        diag = jt >= jpc * ci

                        s2 = ps.tile([P, 2 * CHUNK], F32, tag="s2",
                                     name=f"s{ci}_{hpass}_{jt}", bufs=2)
                        es = sb.tile([P, 2 * CHUNK], BF16, tag="es",
                                     name=f"es{ci}_{hpass}_{jt}", bufs=3)

                        for h2 in range(2):
                            # S^T tile = K_h @ Q_h^T (row-tiled, K=64)
                            nc.tensor.matmul(
                                s2[:, h2 * CHUNK + rel:(h2 + 1) * CHUNK],
                                lhsT=kt_sb[hpass][h2 * DH:(h2 + 1) * DH,
                                                  jt * P:(jt + 1) * P],
                                rhs=qt_sb[hpass][h2 * DH:(h2 + 1) * DH,
                                                 c0 + rel:c0 + CHUNK],
                                start=True, stop=True,
                                tile_position=(h2 * DH, 0))
                        # one exp for both heads (both psum banks)
                        nc.scalar.activation(
                            es.rearrange("p (t c) -> p t c", t=2)[:, :, rel:],
                            s2.rearrange("p (t c) -> p t c", t=2)[:, :, rel:],
                            ActFn.Exp)
                        if diag:
                            for h2 in range(2):
                                nc.vector.tensor_mul(
                                    es[:, h2 * CHUNK + rel:
                                       h2 * CHUNK + rel + P],
                                    es[:, h2 * CHUNK + rel:
                                       h2 * CHUNK + rel + P],
                                    mask_sb[:])
                        for h2 in range(2):
                            h = 2 * hpass + h2
                            # O^T(+sums) accumulation: V_aug^T @ expS^T
                            nc.tensor.matmul(
                                ot_ps[h2][:, rel:CHUNK],
                                lhsT=v_sb[jt][:, h * DHA:(h + 1) * DHA],
                                rhs=es[:, h2 * CHUNK + rel:(h2 + 1) * CHUNK],
                                start=(jt == 0), stop=(jt == jt_end - 1))
                        pop_work(late_ok=(jt >= jt_end - 2))

                    # evacuate psum fast, normalize off the PE critical path
                    for h2 in range(2):
                        h = 2 * hpass + h2
                        otr = sb.tile([DH, CHUNK], F32, tag=f"otr{h2}",
                                      name=f"otr{ci}_{h}", bufs=2)
                        srow = sb.tile([1, CHUNK], F32, tag=f"sr{h2}",
                                       name=f"sr{ci}_{h}", bufs=2)
                        nc.vector.tensor_copy(otr[:], ot_ps[h2][0:DH, :])
                        nc.vector.tensor_copy(srow[:], ot_ps[h2][DH:DHA, :])
                        rcp = sb.tile([1, CHUNK], F32, tag=f"rcp{h2}",
                                      name=f"rcp{ci}_{h}", bufs=2)
                        nc.vector.reciprocal_approx_fast(rcp[:], srow[:])
                        bc_sb = sb.tile([DH, CHUNK], F32, tag=f"bc{h2}",
                                        name=f"bc{ci}_{h}", bufs=2)
                        nc.gpsimd.partition_broadcast(bc_sb[:], rcp[:],
                                                      channels=DH)
                        nc.vector.tensor_mul(
                            ot_sb[hpass][h2 * DH:(h2 + 1) * DH,
                                         c0:c0 + CHUNK],
                            otr[:], bc_sb[:])

                    if last:
                        # this pair's half of the chunk goes out now
                        last_parts.update(emit_ag_pair(ci, hpass))
                    else:
                        nc.sync.dma_start(
                            binf[hpass * P:(hpass + 1) * P, :],
                            ot_sb[hpass][:, c0:c0 + CHUNK])

                if last:
                    agt = dict(last_parts)
                    last_parts = {}
                else:
                    agt = emit_ag_full(ci, binf)
                slices = [(agt[k], 0) for k in range(nko)]
                evens = [k for k in range(nko) if k % 2 == 0]
                odds = [k for k in range(nko) if k % 2 == 1]
                nm = HS // P
                if not last:
                    for m in range(nm):
                        work_late.append(
                            lambda ci=ci, m=m, s=slices, ko=evens + odds:
                            emit_proj(ci, m, s, ko))
                else:
                    # split each output block's projection: the even k-tiles
                    # (from the pair-A AllGather) run while pair-B flies
                    op_tiles = {}

                    def proj_ev(ci, m, s):
                        op_tiles[m] = emit_proj(ci, m, s, evens, evac=False)

                    def proj_od(ci, m, s):
                        emit_proj(ci, m, s, odds, op_ps=op_tiles.pop(m))

                    for m in range(nm):
                        work_late.append(
                            lambda ci=ci, m=m, s=slices: proj_ev(ci, m, s))
                    for m in range(nm):
                        work_late.append(
                            lambda ci=ci, m=m, s=slices: proj_od(ci, m, s))

            while work_early or work_late:
                pop_work(late_ok=True)

    if compile:
        nc.compile()
    return nc


def make_in_maps(x, Wq, Wk, Wv, Wo, n_cores=N_CORES):
    import ml_dtypes
    bf16 = ml_dtypes.bfloat16
    scale = np.float32(DH ** -0.5)
    # band mask for the diagonal j-tile of S^T [j,i]: keep j <= i
    mask_b = np.triu(np.ones((P, P), np.float32)).astype(bf16)
    in_maps = []
    for c in range(n_cores):
        b, r = divmod(c, 4)
        hs = r * HS
        in_maps.append({
            "xT": np.ascontiguousarray(x[b].T).astype(bf16),
            "wq": (Wq[:, hs:hs + HS] * scale).astype(bf16),
            "wk": np.ascontiguousarray(Wk[:, hs:hs + HS]).astype(bf16),
            "wv": np.ascontiguousarray(Wv[:, hs:hs + HS]).astype(bf16),
            "wo": np.ascontiguousarray(Wo[:, hs:hs + HS]).astype(bf16),
            "mask_c": mask_b,
        })
    return in_maps


def assemble_out(results, B, seq, n_cores=N_CORES):
    out = np.empty((B, seq, INNER), np.float32)
    for c in range(n_cores):
        b, r = divmod(c, 4)
        out[b][:, r * HS:(r + 1) * HS] = results[c]["outT"].T
    return out


_NC_CACHE = {}


def kernel(x, Wq, Wk, Wv, Wo):
    from concourse import bass_utils
    x = np.asarray(x, np.float32)
    B, seq, dim = x.shape
    key = (seq, dim)
    if key not in _NC_CACHE:
        _NC_CACHE[key] = build_nc(seq=seq, dim=dim)
    nc = _NC_CACHE[key]
    in_maps = make_in_maps(x, np.asarray(Wq, np.float32),
                           np.asarray(Wk, np.float32),
                           np.asarray(Wv, np.float32),
                           np.asarray(Wo, np.float32))
    res = bass_utils.run_bass_kernel_spmd(
        nc, in_maps, core_ids=list(range(N_CORES)))
    return assemble_out(res.results, B, seq)


# revision 20
# speedup vs baseline: 1.1282x; 1.0273x over previous
"""Distributed causal multi-head attention for one TRN2 chip (8 NeuronCores).

Sharding: batch (2) x head-groups (4 heads/core) -> 8 cores.
Core c handles batch c//4, heads [ (c%4)*4 , (c%4)*4+4 ).
Per core: QKV projections for its 4 heads, flash-style causal attention
with scores kept transposed (S^T = K @ Q^T) so the PV product needs no
transposes; V is augmented with a ones column so the softmax denominators
fall out of the same matmul (row 64 of each head's O^T psum).  Then an
AllGather of the attention output (pre-Wo, 4-core group = one batch) and
a column-sliced output projection.  Host assembles the 8 column/batch
shards.  Compute dtype bf16 (PSUM accumulation fp32), softmax in fp32.

Scheduling: the attention loop keeps the in-order PE queue saturated by
interleaving KT/V projections for later chunks and the AllGather-gated
output projections as queued work items (so the HAM clock gate never
re-throttles).  Chunk 0+1 share one AllGather (it completes under chunk
2's attention); the last chunk's AllGather is split per head-pair so the
first half overlaps pass B and the output projection can start on half
the k-tiles while the second half is still in flight.
"""

import sys
from collections import deque

import numpy as np

sys.path.insert(0, "/opt/trn_rl_repo")

import concourse.bass as bass  # noqa: E402
import concourse.bacc as bacc  # noqa: E402
import concourse.tile as tile  # noqa: E402
import concourse.mybir as mybir  # noqa: E402

F32 = mybir.dt.float32
BF16 = mybir.dt.bfloat16
ActFn = mybir.ActivationFunctionType

P = 128          # partition dim
CHUNK = 512      # i-chunk (matmul moving free dim, one psum bank of fp32)
DH = 64          # head dim
HPC = 4          # heads per core
HS = HPC * DH    # 256 per-core inner slice
DHA = DH + 1     # augmented head dim (ones column for softmax sums)
INNER = 1024     # total inner dim (16 heads x 64)
N_CORES = 8
GROUPS = [[0, 1, 2, 3], [4, 5, 6, 7]]


def build_nc(seq=2048, dim=1024, n_cores=N_CORES, groups=GROUPS, compile=True):
    """Build the SPMD Bass graph (identical on all cores)."""
    nch = seq // CHUNK          # i-chunks
    jpc = CHUNK // P            # j-tiles per chunk (4)
    njt = seq // P              # j-tiles
    nk = dim // P               # feature k-tiles
    nko = INNER // P            # inner k-tiles for the output projection
    grp = len(groups[0])        # replica group size (4)

    nc = bacc.Bacc("TRN2", target_bir_lowering=False, debug=False,
                   enable_asserts=False, num_devices=n_cores)

    xT = nc.dram_tensor("xT", [dim, seq], BF16, kind="ExternalInput").ap()
    wq = nc.dram_tensor("wq", [dim, HS], BF16, kind="ExternalInput").ap()
    wk = nc.dram_tensor("wk", [dim, HS], BF16, kind="ExternalInput").ap()
    wv = nc.dram_tensor("wv", [dim, HS], BF16, kind="ExternalInput").ap()
    wo = nc.dram_tensor("wo", [INNER, HS], BF16, kind="ExternalInput").ap()
    mask_c = nc.dram_tensor("mask_c", [P, P], BF16, kind="ExternalInput").ap()
    outT = nc.dram_tensor("outT", [HS, seq], F32, kind="ExternalOutput").ap()

    with tile.TileContext(nc) as tc:
        with tc.tile_pool(name="sb", bufs=1) as sb, \
             tc.tile_pool(name="ps", bufs=1, space="PSUM") as ps, \
             tc.tile_pool(name="dram", bufs=1, space="DRAM") as dram:

            # ---- load inputs ----
            xt = [sb.tile([P, seq], BF16, tag=f"xt{k}", name=f"xt{k}")
                  for k in range(nk)]
            wq_sb = [sb.tile([P, HS], BF16, tag=f"wq{k}", name=f"wq{k}")
                     for k in range(nk)]
            wk_sb = [sb.tile([P, HS], BF16, tag=f"wk{k}", name=f"wk{k}")
                     for k in range(nk)]
            wv_sb = [sb.tile([P, HS], BF16, tag=f"wv{k}", name=f"wv{k}")
                     for k in range(nk)]
            wo_sb = [sb.tile([P, HS], BF16, tag=f"wo{k}", name=f"wo{k}")
                     for k in range(nko)]
            mask_sb = sb.tile([P, P], BF16, tag="mask", name="mask")

            # inputs spread across engine DMA queues so the loads issue
            # in parallel (the sync queue alone serializes ~40 descriptors)
            for k in range(nk):
                nc.sync.dma_start(xt[k][:], xT[k * P:(k + 1) * P, :])
                nc.scalar.dma_start(wq_sb[k][:], wq[k * P:(k + 1) * P, :])
                nc.scalar.dma_start(wk_sb[k][:], wk[k * P:(k + 1) * P, :])
                nc.gpsimd.dma_start(wv_sb[k][:], wv[k * P:(k + 1) * P, :])
            for k in range(nko):
                nc.gpsimd.dma_start(wo_sb[k][:], wo[k * P:(k + 1) * P, :])
            nc.gpsimd.dma_start(mask_sb[:], mask_c[:])

            # warm up the collectives firmware while QKV runs
            warm_in = dram.tile([P, 4], BF16, tag="warm_i", name="warm_i")
            warm_out = dram.tile([grp * P, 4], BF16,
                                 tag="warm_o", name="warm_o")
            nc.sync.dma_start(warm_in[:], mask_c[0:P, 0:4])
            nc.gpsimd.collective_compute(
                "AllGather", mybir.AluOpType.bypass, replica_groups=groups,
                ins=[warm_in.opt()], outs=[warm_out.opt()])

            # persistent QKV results
            qt_sb = [sb.tile([P, seq], BF16, tag=f"qt{p}", name=f"qt{p}")
                     for p in range(2)]
            kt_sb = [sb.tile([P, seq], BF16, tag=f"kt{p}", name=f"kt{p}")
                     for p in range(2)]
            v_sb = [sb.tile([P, HPC * DHA], BF16, tag=f"v{j}", name=f"v{j}")
                    for j in range(njt)]
            ot_sb = [sb.tile([P, seq], BF16, tag=f"ot{p}", name=f"ot{p}")
                     for p in range(2)]

            # ---- interleavable work items (each emits one psum group) ----
            def emit_kt(pair, ch):
                pt = ps.tile([P, CHUNK], F32, tag="misc",
                             name=f"ktps{pair}_{ch}", bufs=2)
                for k in range(nk):
                    nc.tensor.matmul(
                        pt[:], lhsT=wk_sb[k][:, pair * P:(pair + 1) * P],
                        rhs=xt[k][:, ch * CHUNK:(ch + 1) * CHUNK],
                        start=(k == 0), stop=(k == nk - 1))
                nc.scalar.activation(
                    kt_sb[pair][:, ch * CHUNK:(ch + 1) * CHUNK], pt[:],
                    ActFn.Copy)

            def emit_v(jt):
                pt = ps.tile([P, HS], F32, tag="misc",
                             name=f"vps{jt}", bufs=2)
                for k in range(nk):
                    nc.tensor.matmul(
                        pt[:], lhsT=xt[k][:, jt * P:(jt + 1) * P],
                        rhs=wv_sb[k][:],
                        start=(k == 0), stop=(k == nk - 1))
                nc.scalar.activation(
                    v_sb[jt].rearrange("p (h d) -> p h d", h=HPC)[:, :, 0:DH],
                    pt.rearrange("p (h d) -> p h d", h=HPC), ActFn.Copy)
                nc.vector.memset(
                    v_sb[jt].rearrange("p (h d) -> p h d", h=HPC)[:, :, DH:DHA],
                    1.0)

            def emit_proj(ci, m, slices, korder, op_ps=None, evac=True):
                # transposed output block: outT[m*128:(m+1)*128, chunk ci]
                # = Wo[:, m-slice].T @ attT[:, chunk] over the k-tiles in
                # `korder` (a partial pass keeps op_ps alive).  The moving
                # operand is the gathered attT tile (N=512, half the
                # matmuls of the untransposed form); the host untransposes.
                c0 = ci * CHUNK
                first = op_ps is None
                if first:
                    op_ps = ps.tile([P, CHUNK], F32, tag="misc",
                                    name=f"op{ci}_{m}", bufs=2)
                for n, k in enumerate(korder):
                    ag_t, coff = slices[k]
                    nc.tensor.matmul(
                        op_ps[:],
                        lhsT=wo_sb[k][:, m * P:(m + 1) * P],
                        rhs=ag_t[:, coff:coff + CHUNK],
                        start=(first and n == 0),
                        stop=(evac and n == len(korder) - 1))
                if not evac:
                    return op_ps
                o_sb = sb.tile([P, CHUNK], F32, tag="osb",
                               name=f"o{ci}_{m}", bufs=2)
                nc.vector.tensor_copy(o_sb[:], op_ps[:])
                nc.sync.dma_start(
                    outT[m * P:(m + 1) * P, c0:c0 + CHUNK], o_sb[:])
                return None

            work_early = deque()   # KT/V for future chunks (not gated)
            work_late = deque()    # output projections (gated on AllGather)

            def pop_work(late_ok):
                if work_early:
                    work_early.popleft()()
                    if len(work_early) > 4:
                        work_early.popleft()()
                elif late_ok and work_late:
                    work_late.popleft()()

            def emit_ag_full(ci, bounce_in):
                # one AllGather for both head pairs of chunk `ci` (256KB —
                # amortizes the ncfw floor; rank-major rows land so that
                # gathered row-block k*128 is exactly attT k-tile k)
                bounce_out = dram.tile([grp * 2 * P, CHUNK], BF16,
                                       tag="boutf", name=f"boutf{ci}", bufs=2)
                nc.gpsimd.collective_compute(
                    "AllGather", mybir.AluOpType.bypass,
                    replica_groups=groups,
                    ins=[bounce_in.opt()], outs=[bounce_out.opt()])
                tiles = {}
                for k in range(nko):
                    t = sb.tile([P, CHUNK], BF16, tag=f"ag{k}",
                                name=f"ag{ci}_{k}", bufs=2)
                    nc.sync.dma_start(t[:],
                                      bounce_out[k * P:(k + 1) * P, :])
                    tiles[k] = t
                return tiles

            def emit_ag_pair(ci, pair):
                # half AllGather (one head pair) of the chunk `ci` — fired
                # right after that pair's normalize, so pair A overlaps the
                # second attention pass and both stay small (cheap on CC).
                # The gathered k-tiles (2r+pair) load right behind it.
                c0 = ci * CHUNK
                bounce_in = dram.tile([P, CHUNK], BF16, tag=f"binh{pair}",
                                      name=f"binh{ci}_{pair}", bufs=2)
                bounce_out = dram.tile([grp * P, CHUNK], BF16,
                                       tag=f"bouth{pair}",
                                       name=f"bouth{ci}_{pair}", bufs=2)
                nc.sync.dma_start(bounce_in[:], ot_sb[pair][:, c0:c0 + CHUNK])
                nc.gpsimd.collective_compute(
                    "AllGather", mybir.AluOpType.bypass,
                    replica_groups=groups,
                    ins=[bounce_in.opt()], outs=[bounce_out.opt()])
                tiles = {}
                for r in range(grp):
                    k = 2 * r + pair
                    t = sb.tile([P, CHUNK], BF16, tag=f"ag{k}",
                                name=f"ag{ci}_{k}", bufs=2)
                    nc.sync.dma_start(t[:], bounce_out[r * P:(r + 1) * P, :])
                    tiles[k] = t
                return tiles

            # ---- upfront projections: chunk-0/1 Q, chunk-0 K, chunk-0 V
            # (Q for later chunks is deferred into the work queue).  Q is
            # k-outer / weight-stationary so it pipelines with the xT DMAs;
            # two chunks share one 2-bank psum tile.
            def emit_qt(pair, chh):
                wch = min(2, nch - chh)
                pt = ps.tile([P, wch * CHUNK], F32, tag="s2",
                             name=f"qps{pair}_{chh}", bufs=2)
                for k in range(nk):
                    for c in range(wch):
                        nc.tensor.matmul(
                            pt[:, c * CHUNK:(c + 1) * CHUNK],
                            lhsT=wq_sb[k][:, pair * P:(pair + 1) * P],
                            rhs=xt[k][:, (chh + c) * CHUNK:
                                       (chh + c + 1) * CHUNK],
                            start=(k == 0), stop=(k == nk - 1))
                nc.scalar.activation(
                    qt_sb[pair][:, chh * CHUNK:(chh + wch) * CHUNK],
                    pt[:], ActFn.Copy)

            for pair in range(2):
                emit_qt(pair, 0)
            for pair in range(2):
                emit_kt(pair, 0)
            for jt in range(jpc):
                emit_v(jt)
            for chh in range(2, nch, 2):
                for pair in range(2):
                    work_early.append(
                        lambda pair=pair, chh=chh: emit_qt(pair, chh))

            # ---- attention chunks ----
            last_parts = {}
            for ci in range(nch):
                jt_end = jpc * (ci + 1)
                c0 = ci * CHUNK
                last = ci == nch - 1

                if ci + 1 < nch:
                    for pair in range(2):
                        work_early.append(
                            lambda pair=pair, ch=ci + 1: emit_kt(pair, ch))
                    for jt in range(jpc * (ci + 1), jpc * (ci + 2)):
                        work_early.append(lambda jt=jt: emit_v(jt))

                binf = None if last else dram.tile(
                    [2 * P, CHUNK], BF16, tag="binf", name=f"binf{ci}",
                    bufs=2)

                for hpass in range(2):
                    # heads 2*hpass, 2*hpass+1  (== head pair `hpass`)
                    ot_ps = [ps.tile([DHA, CHUNK], F32, tag=f"ot{h2}",
                                     name=f"ot{ci}_{hpass}_{h2}", bufs=1)
                             for h2 in range(2)]
                    for jt in range(jt_end):
                        rel = max(0, (jt - jpc * ci)) * P
                        diag = jt >= jpc * ci

                        s2 = ps.tile([P, 2 * CHUNK], F32, tag="s2",
                                     name=f"s{ci}_{hpass}_{jt}", bufs=2)
                        es = sb.tile([P, 2 * CHUNK], BF16, tag="es",
                                     name=f"es{ci}_{hpass}_{jt}", bufs=3)

                        for h2 in range(2):
                            # S^T tile = K_h @ Q_h^T (row-tiled, K=64)
                            nc.tensor.matmul(
                                s2[:, h2 * CHUNK + rel:(h2 + 1) * CHUNK],
                                lhsT=kt_sb[hpass][h2 * DH:(h2 + 1) * DH,
                                                  jt * P:(jt + 1) * P],
                                rhs=qt_sb[hpass][h2 * DH:(h2 + 1) * DH,
                                                 c0 + rel:c0 + CHUNK],
                                start=True, stop=True,
                                tile_position=(h2 * DH, 0))
                        # one exp for both heads (both psum banks)
                        nc.scalar.activation(
                            es.rearrange("p (t c) -> p t c", t=2)[:, :, rel:],
                            s2.rearrange("p (t c) -> p t c", t=2)[:, :, rel:],
                            ActFn.Exp)
                        if diag:
                            for h2 in range(2):
                                nc.vector.tensor_mul(
                                    es[:, h2 * CHUNK + rel:
                                       h2 * CHUNK + rel + P],
                                    es[:, h2 * CHUNK + rel:
                                       h2 * CHUNK + rel + P],
                                    mask_sb[:])
                        for h2 in range(2):
                            h = 2 * hpass + h2
                            # O^T(+sums) accumulation: V_aug^T @ expS^T
                            nc.tensor.matmul(
                                ot_ps[h2][:, rel:CHUNK],
                                lhsT=v_sb[jt][:, h * DHA:(h + 1) * DHA],
                                rhs=es[:, h2 * CHUNK + rel:(h2 + 1) * CHUNK],
                                start=(jt == 0), stop=(jt == jt_end - 1))
                        pop_work(late_ok=(hpass == 1 and
                                          jt >= jt_end - 2))

                    # evacuate psum fast, normalize off the PE critical path
                    for h2 in range(2):
                        h = 2 * hpass + h2
                        otr = sb.tile([DH, CHUNK], F32, tag=f"otr{h2}",
                                      name=f"otr{ci}_{h}", bufs=2)
                        srow = sb.tile([1, CHUNK], F32, tag=f"sr{h2}",
                                       name=f"sr{ci}_{h}", bufs=2)
                        nc.vector.tensor_copy(otr[:], ot_ps[h2][0:DH, :])
                        nc.vector.tensor_copy(srow[:], ot_ps[h2][DH:DHA, :])
                        rcp = sb.tile([1, CHUNK], F32, tag=f"rcp{h2}",
                                      name=f"rcp{ci}_{h}", bufs=2)
                        nc.vector.reciprocal_approx_fast(rcp[:], srow[:])
                        bc_sb = sb.tile([DH, CHUNK], F32, tag=f"bc{h2}",
                                        name=f"bc{ci}_{h}", bufs=2)
                        nc.gpsimd.partition_broadcast(bc_sb[:], rcp[:],
                                                      channels=DH)
                        nc.vector.tensor_mul(
                            ot_sb[hpass][h2 * DH:(h2 + 1) * DH,
                                         c0:c0 + CHUNK],
                            otr[:], bc_sb[:])

                    if last:
                        # this pair's half of the chunk goes out now
                        last_parts.update(emit_ag_pair(ci, hpass))
                    else:
                        nc.sync.dma_start(
                            binf[hpass * P:(hpass + 1) * P, :],
                            ot_sb[hpass][:, c0:c0 + CHUNK])

                if last:
                    agt = dict(last_parts)
                    last_parts = {}
                else:
                    agt = emit_ag_full(ci, binf)
                slices = [(agt[k], 0) for k in range(nko)]
                evens = [k for k in range(nko) if k % 2 == 0]
                odds = [k for k in range(nko) if k % 2 == 1]
                nm = HS // P
                if not last:
                    for m in range(nm):
                        work_late.append(
                            lambda ci=ci, m=m, s=slices, ko=evens + odds:
                            emit_proj(ci, m, s, ko))
                else:
                    # split each output block's projection: the even k-tiles
                    # (from the pair-A AllGather) run while pair-B flies
                    op_tiles = {}

                    def proj_ev(ci, m, s):
                        op_tiles[m] = emit_proj(ci, m, s, evens, evac=False)

                    def proj_od(ci, m, s):
                        emit_proj(ci, m, s, odds, op_ps=op_tiles.pop(m))

                    for m in range(nm):
                        work_late.append(
                            lambda ci=ci, m=m, s=slices: proj_ev(ci, m, s))
                    for m in range(nm):
                        work_late.append(
                            lambda ci=ci, m=m, s=slices: proj_od(ci, m, s))

            while work_early or work_late:
                pop_work(late_ok=True)

    if compile:
        nc.compile()
    return nc


def make_in_maps(x, Wq, Wk, Wv, Wo, n_cores=N_CORES):
    import ml_dtypes
    bf16 = ml_dtypes.bfloat16
    scale = np.float32(DH ** -0.5)
    # band mask for the diagonal j-tile of S^T [j,i]: keep j <= i
    mask_b = np.triu(np.ones((P, P), np.float32)).astype(bf16)
    in_maps = []
    for c in range(n_cores):
        b, r = divmod(c, 4)
        hs = r * HS
        in_maps.append({
            "xT": np.ascontiguousarray(x[b].T).astype(bf16),
            "wq": (Wq[:, hs:hs + HS] * scale).astype(bf16),
            "wk": np.ascontiguousarray(Wk[:, hs:hs + HS]).astype(bf16),
            "wv": np.ascontiguousarray(Wv[:, hs:hs + HS]).astype(bf16),
            "wo": np.ascontiguousarray(Wo[:, hs:hs + HS]).astype(bf16),
            "mask_c": mask_b,
        })
    return in_maps


def assemble_out(results, B, seq, n_cores=N_CORES):
    out = np.empty((B, seq, INNER), np.float32)
    for c in range(n_cores):
        b, r = divmod(c, 4)
        out[b][:, r * HS:(r + 1) * HS] = results[c]["outT"].T
    return out


_NC_CACHE = {}


def kernel(x, Wq, Wk, Wv, Wo):
    from concourse import bass_utils
    x = np.asarray(x, np.float32)
    B, seq, dim = x.shape
    key = (seq, dim)
    if key not in _NC_CACHE:
        _NC_CACHE[key] = build_nc(seq=seq, dim=dim)
    nc = _NC_CACHE[key]
    in_maps = make_in_maps(x, np.asarray(Wq, np.float32),
                           np.asarray(Wk, np.float32),
                           np.asarray(Wv, np.float32),
                           np.asarray(Wo, np.float32))
    res = bass_utils.run_bass_kernel_spmd(
        nc, in_maps, core_ids=list(range(N_CORES)))
    return assemble_out(res.results, B, seq)
